# revision 1
# baseline (speedup 1.0000x reference)
"""Block Hadamard transform (128-wide blocks) on 8 Trainium2 NeuronCores.

y[..., n*128:(n+1)*128] = x[..., n*128:(n+1)*128] @ H  for the fixed
128x128 (already 1/sqrt(128)-scaled) Hadamard matrix H.

Strategy: the op is a uniform (rows, 128) @ (128, 128) matmul after viewing
x as block-rows of 128 contiguous elements.  Data-parallel shard of the
block-rows across 8 cores; H replicated.  Per core, per 128x128 tile:
  PE transpose (fp32) -> PSUM->SBUF copy -> fp32 matmul vs stationary-free
  moving H -> PSUM->SBUF copy -> contiguous DMA out.
All DMAs move >=1MiB contiguous slabs (512B per partition per descriptor).
"""

import numpy as np

import concourse.bass as bass  # noqa: F401  (registers engines)
import concourse.mybir as mybir
import concourse.tile as tile
from concourse import bacc
from concourse.bass_utils import run_bass_kernel_spmd
from concourse.masks import make_identity

N_CORES = 8
P = 128
FULL_SHAPE = (4, 4096, 4096)
S_TOTAL = int(np.prod(FULL_SHAPE)) // P  # 524288 block-rows
S = S_TOTAL // N_CORES                   # 65536 block-rows per core
CH = 16                                  # 128-row tiles per supertile (1 MiB DMA)
NSUPER = S // (P * CH)                   # 32

F32 = mybir.dt.float32

_CACHE: dict = {}


def _build(
    repeat: int = 1,
    ch: int = CH,
    xbufs: int = 3,
    ybufs: int = 3,
    tbufs: int = 4,
    psbufs: int = 3,
    out_dma_engine: str = "sync",
    in_dma_engine: str = "scalar",
    layout: str = "interleave",
    split_dma: bool = True,
    group: int = 1,
    loop_repeat: int = 1,
    sp_in: int | None = None,
    dma_mode: str = "hybrid",  # split | alt | quarter | hybrid
):
    nsuper = S // (P * ch)
    nc = bacc.Bacc(
        "TRN2", target_bir_lowering=False, debug=False, num_devices=N_CORES
    )
    xs = nc.dram_tensor("xs", [S, P], F32, kind="ExternalInput")
    hh = nc.dram_tensor("h", [P, P], F32, kind="ExternalInput")
    ys = nc.dram_tensor("ys", [S, P], F32, kind="ExternalOutput")

    with tile.TileContext(nc) as tc:
        in_eng = getattr(nc, in_dma_engine)
        out_eng = getattr(nc, out_dma_engine)
        with (
            tc.tile_pool(name="consts", bufs=1) as consts,
            tc.tile_pool(name="xsup", bufs=xbufs) as xsup_pool,
            tc.tile_pool(name="ysup", bufs=ybufs) as ysup_pool,
            tc.tile_pool(name="tsb", bufs=tbufs) as tsb_pool,
            tc.tile_pool(name="tpsum", bufs=psbufs, space="PSUM") as tpsum_pool,
            tc.tile_pool(name="ypsum", bufs=psbufs, space="PSUM") as ypsum_pool,
        ):
            identity = consts.tile([P, P], F32)
            make_identity(nc, identity[:])
            h_sb = consts.tile([P, P], F32)
            nc.sync.dma_start(h_sb[:], hh[:, :])

            # "interleave": partition p of tile j holds block-row j*128+p
            #   (16 strided 512B descriptors per partition per supertile).
            # "chunk": partition p holds block-rows [p*ch, (p+1)*ch) of the
            #   supertile (one contiguous ch*512B descriptor per partition);
            #   PE tiles then cover row sets {k*ch + j : k} which is fine
            #   since block-rows are independent.
            pattern = "(j p) f -> p j f" if layout == "interleave" else "(p j) f -> p j f"

            import contextlib

            loop_cm = (
                tc.For_i(0, loop_repeat, 1)
                if loop_repeat > 1
                else contextlib.nullcontext()
            )
            with loop_cm:
                body(
                    nc, tc, xs, ys, xsup_pool, ysup_pool, tsb_pool,
                    tpsum_pool, ypsum_pool, identity, h_sb,
                    nsuper, repeat, ch, group, pattern, split_dma,
                    in_eng, out_eng, sp_in if sp_in is not None else ch // 2,
                    dma_mode,
                )

    nc.compile()
    return nc


def body(
    nc, tc, xs, ys, xsup_pool, ysup_pool, tsb_pool,
    tpsum_pool, ypsum_pool, identity, h_sb,
    nsuper, repeat, ch, group, pattern, split_dma,
    in_eng, out_eng, sp_in, dma_mode="split",
):
    qs = (nc.sync, nc.scalar)
    for i in range(nsuper * repeat):
        i = i % nsuper
        rows = slice(i * ch * P, (i + 1) * ch * P)
        xt = xsup_pool.tile([P, ch, P], F32)
        src = xs[rows, :].rearrange(pattern, p=P)
        if dma_mode == "alt":
            qs[i % 2].dma_start(xt[:], src)
        elif dma_mode == "hybrid":
            half = ch // 2
            nc.sync.dma_start(xt[:, :half, :], src[:, :half, :])
            nc.scalar.dma_start(xt[:, half:, :], src[:, half:, :])
        elif dma_mode == "quarter":
            q = ch // 4
            for piece in range(4):
                sl = slice(piece * q, (piece + 1) * q)
                qs[piece % 2].dma_start(xt[:, sl, :], src[:, sl, :])
        elif split_dma:
            half = sp_in
            nc.sync.dma_start(xt[:, :half, :], src[:, :half, :])
            nc.scalar.dma_start(xt[:, half:, :], src[:, half:, :])
        else:
            in_eng.dma_start(xt[:], src)
        yt = ysup_pool.tile([P, ch, P], F32)
        for g in range(ch // group):
            # `group` 128x128 tiles share one PSUM bank (group*512B
            # per partition <= 2KB for group<=4) so the PSUM->SBUF
            # copies amortize per-instruction overhead.
            tp = tpsum_pool.tile([P, group, P], F32)
            for k in range(group):
                nc.tensor.transpose(
                    tp[:, k, :], xt[:, g * group + k, :], identity[:]
                )
            tsb = tsb_pool.tile([P, group, P], F32)
            if g % 2 == 0:
                nc.scalar.copy(tsb[:], tp[:])
            else:
                nc.vector.tensor_copy(tsb[:], tp[:])
            yp = ypsum_pool.tile([P, group, P], F32)
            for k in range(group):
                nc.tensor.matmul(
                    yp[:, k, :], tsb[:, k, :], h_sb[:],
                    start=True, stop=True,
                )
            ysl = yt[:, g * group : (g + 1) * group, :]
            if g % 2 == 0:
                nc.vector.tensor_copy(ysl, yp[:])
            else:
                nc.scalar.copy(ysl, yp[:])
        dst = ys[rows, :].rearrange(pattern, p=P)
        if dma_mode in ("alt", "hybrid"):
            qs[(i + 1) % 2].dma_start(dst, yt[:])
        elif dma_mode == "quarter":
            q = ch // 4
            for piece in range(4):
                sl = slice(piece * q, (piece + 1) * q)
                qs[(piece + 1) % 2].dma_start(dst[:, sl, :], yt[:, sl, :])
        elif split_dma:
            half = ch - sp_in
            nc.sync.dma_start(dst[:, :half, :], yt[:, :half, :])
            nc.scalar.dma_start(dst[:, half:, :], yt[:, half:, :])
        else:
            out_eng.dma_start(dst, yt[:])


DEFAULT_CFG: dict = {}


def _get_nc():
    if "nc" not in _CACHE:
        _CACHE["nc"] = _build(**DEFAULT_CFG)
    return _CACHE["nc"]


def _run(x: np.ndarray, H: np.ndarray, trace: bool = False):
    nc = _get_nc()
    x_flat = np.ascontiguousarray(
        np.asarray(x, dtype=np.float32).reshape(S_TOTAL, P)
    )
    h_np = np.ascontiguousarray(np.asarray(H, dtype=np.float32))
    in_maps = [
        {"xs": x_flat[k * S : (k + 1) * S], "h": h_np} for k in range(N_CORES)
    ]
    try:
        res = run_bass_kernel_spmd(
            nc, in_maps, core_ids=list(range(N_CORES)), trace=trace
        )
    except ModuleNotFoundError:
        # This axon build has no NTFF profile hook (antenv.axon_hooks); if
        # tracing was requested via env (BASS_TRACE), fall back to untraced.
        import os

        os.environ["BASS_NEVER_TRACE"] = "1"
        res = run_bass_kernel_spmd(
            nc, in_maps, core_ids=list(range(N_CORES)), trace=False
        )
    y = np.concatenate([res.results[k]["ys"] for k in range(N_CORES)], axis=0)
    return y.reshape(FULL_SHAPE).astype(np.float32, copy=False), res


def kernel(x: np.ndarray, H: np.ndarray) -> np.ndarray:
    y, _ = _run(x, H, trace=False)
    return y


if __name__ == "__main__":
    rng = np.random.default_rng(0)
    x = rng.standard_normal(FULL_SHAPE, dtype=np.float32)

    def _hadamard(n):
        h = np.array([[1.0]], dtype=np.float32)
        while h.shape[0] < n:
            h = np.block([[h, h], [h, -h]])
        return h

    H = (_hadamard(P) / np.sqrt(P)).astype(np.float32)
    y = kernel(x, H)
    expected = (x.reshape(-1, P) @ H).reshape(FULL_SHAPE)
    err = np.max(np.abs(y - expected)) / np.max(np.abs(expected))
    print("self-check rel err:", err)



# revision 2
# speedup vs baseline: 2.2607x; 2.2607x over previous
"""Block Hadamard transform (128-wide blocks) on 8 Trainium2 NeuronCores.

y[..., n*128:(n+1)*128] = x[..., n*128:(n+1)*128] @ H  for the fixed
128x128 (already 1/sqrt(128)-scaled) Hadamard matrix H.

Strategy: the op is a uniform (rows, 128) @ (128, 128) matmul after viewing
x as block-rows of 128 contiguous elements.  Data-parallel shard of the
block-rows across 8 cores; H replicated.  Per core, per 128x128 tile:
  PE transpose (fp32) -> PSUM->SBUF cast-copy to bf16 -> bf16 matmul vs
  moving bf16 H -> PSUM->SBUF fp32 copy -> contiguous DMA out.
DMA uses a "chunk" layout: partition p of a supertile holds ch consecutive
block-rows, so each 128-partition DMA moves ch*512B contiguous per
partition (>= 8KB descriptors) at near-peak HBM bandwidth.  Input DMAs ride
the SP HWDGE ring, output DMAs the ACT ring, so the two directions never
head-of-line block each other.
"""

import numpy as np

import concourse.bass as bass  # noqa: F401  (registers engines)
import concourse.mybir as mybir
import concourse.tile as tile
from concourse import bacc
from concourse.bass_utils import run_bass_kernel_spmd
from concourse.masks import make_identity

N_CORES = 8
P = 128
FULL_SHAPE = (4, 4096, 4096)
S_TOTAL = int(np.prod(FULL_SHAPE)) // P  # 524288 block-rows
S = S_TOTAL // N_CORES                   # 65536 block-rows per core

F32 = mybir.dt.float32
BF16 = mybir.dt.bfloat16

_CACHE: dict = {}


def _build(
    ch: int = 32,       # 128-row tiles per supertile (32 -> 2 MiB DMAs)
    group: int = 4,     # tiles per PSUM bank (4 x 512B = one 2KB bank)
    xbufs: int = 3,
    ybufs: int = 3,
    tbufs: int = 4,
    tpbufs: int = 3,
    ypbufs: int = 3,
    loop_repeat: int = 1,
    mode: str = "v2",
    tcopy_eng: str = "vector",
    ycopy_eng: str = "scalar",
):
    nsuper = S // (P * ch)
    nc = bacc.Bacc(
        "TRN2", target_bir_lowering=False, debug=False, num_devices=N_CORES
    )
    xs = nc.dram_tensor("xs", [S, P], F32, kind="ExternalInput")
    hh = nc.dram_tensor("h", [P, P], F32, kind="ExternalInput")
    ys = nc.dram_tensor("ys", [S, P], F32, kind="ExternalOutput")

    with tile.TileContext(nc) as tc:
        with (
            tc.tile_pool(name="consts", bufs=1) as consts,
            tc.tile_pool(name="xsup", bufs=xbufs) as xsup_pool,
            tc.tile_pool(name="ysup", bufs=ybufs) as ysup_pool,
            tc.tile_pool(name="tsb", bufs=tbufs) as tsb_pool,
            tc.tile_pool(name="tpsum", bufs=tpbufs, space="PSUM") as tpsum_pool,
            tc.tile_pool(name="ypsum", bufs=ypbufs, space="PSUM") as ypsum_pool,
        ):
            identity = consts.tile([P, P], F32)
            make_identity(nc, identity[:])
            h_sb = consts.tile([P, P], F32)
            nc.sync.dma_start(h_sb[:], hh[:, :])
            h_bf = consts.tile([P, P], BF16)
            nc.scalar.copy(h_bf[:], h_sb[:])

            import contextlib

            loop_cm = (
                tc.For_i(0, loop_repeat, 1)
                if loop_repeat > 1
                else contextlib.nullcontext()
            )
            with loop_cm:
                body_v2(
                    nc, xs, ys, xsup_pool, ysup_pool, tsb_pool,
                    tpsum_pool, ypsum_pool, identity, h_bf,
                    nsuper, ch, group, tcopy_eng, ycopy_eng,
                )

    nc.compile()
    return nc


def body_v2(
    nc, xs, ys, xsup_pool, ysup_pool, tsb_pool,
    tpsum_pool, ypsum_pool, identity, h_bf,
    nsuper, ch, group, tcopy_eng, ycopy_eng,
):
    # partition p of supertile i holds block-rows [i*ch*P + p*ch, +ch): each
    # DMA descriptor is ch*512B contiguous per partition.
    pattern = "(p j) f -> p j f"
    teng = getattr(nc, tcopy_eng)
    yeng = getattr(nc, ycopy_eng)
    for i in range(nsuper):
        rows = slice(i * ch * P, (i + 1) * ch * P)
        xt = xsup_pool.tile([P, ch, P], F32)
        nc.sync.dma_start(xt[:], xs[rows, :].rearrange(pattern, p=P))
        yt = ysup_pool.tile([P, ch, P], F32)
        for g in range(ch // group):
            tp = tpsum_pool.tile([P, group, P], F32)
            for k in range(group):
                nc.tensor.transpose(
                    tp[:, k, :], xt[:, g * group + k, :], identity[:]
                )
            tsb = tsb_pool.tile([P, group, P], BF16)
            if tcopy_eng == "scalar":
                teng.copy(tsb[:], tp[:])
            else:
                teng.tensor_copy(tsb[:], tp[:])
            yp = ypsum_pool.tile([P, group, P], F32)
            for k in range(group):
                nc.tensor.matmul(
                    yp[:, k, :], tsb[:, k, :], h_bf[:],
                    start=True, stop=True,
                )
            ysl = yt[:, g * group : (g + 1) * group, :]
            if ycopy_eng == "scalar":
                yeng.copy(ysl, yp[:])
            else:
                yeng.tensor_copy(ysl, yp[:])
        nc.scalar.dma_start(ys[rows, :].rearrange(pattern, p=P), yt[:])


DEFAULT_CFG: dict = {}


def _get_nc():
    if "nc" not in _CACHE:
        _CACHE["nc"] = _build(**DEFAULT_CFG)
    return _CACHE["nc"]


def _run(x: np.ndarray, H: np.ndarray, trace: bool = False):
    nc = _get_nc()
    x_flat = np.ascontiguousarray(
        np.asarray(x, dtype=np.float32).reshape(S_TOTAL, P)
    )
    h_np = np.ascontiguousarray(np.asarray(H, dtype=np.float32))
    in_maps = [
        {"xs": x_flat[k * S : (k + 1) * S], "h": h_np} for k in range(N_CORES)
    ]
    try:
        res = run_bass_kernel_spmd(
            nc, in_maps, core_ids=list(range(N_CORES)), trace=trace
        )
    except ModuleNotFoundError:
        # This axon build has no NTFF profile hook (antenv.axon_hooks); if
        # tracing was requested via env (BASS_TRACE), fall back to untraced.
        import os

        os.environ["BASS_NEVER_TRACE"] = "1"
        res = run_bass_kernel_spmd(
            nc, in_maps, core_ids=list(range(N_CORES)), trace=False
        )
    y = np.concatenate([res.results[k]["ys"] for k in range(N_CORES)], axis=0)
    return y.reshape(FULL_SHAPE).astype(np.float32, copy=False), res


def kernel(x: np.ndarray, H: np.ndarray) -> np.ndarray:
    y, _ = _run(x, H, trace=False)
    return y


if __name__ == "__main__":
    rng = np.random.default_rng(0)
    x = rng.standard_normal(FULL_SHAPE, dtype=np.float32)

    def _hadamard(n):
        h = np.array([[1.0]], dtype=np.float32)
        while h.shape[0] < n:
            h = np.block([[h, h], [h, -h]])
        return h

    H = (_hadamard(P) / np.sqrt(P)).astype(np.float32)
    y = kernel(x, H)
    expected = (x.reshape(-1, P) @ H).reshape(FULL_SHAPE)
    err = np.max(np.abs(y - expected)) / np.max(np.abs(expected))
    print("self-check rel err:", err)
